# revision 1
# baseline (speedup 1.0000x reference)
"""Trainium2 Bass kernel for nn_DialogueGCNModel (DialogueGCN forward).

Strategy (data-parallel over dialogues, 4 dialogues per core):
  - Edges never cross dialogues (windowed construction), so the RGCN
    scatter/gather is reformulated as dense per-dialogue banded-adjacency
    matmuls: agg^T = (sum_r xr_r^T @ A_r^T) * (1/deg), with exact 0/1
    adjacency masks shipped as fp8 and the degree scaling applied in f32.
  - Everything on-device is dense PE matmuls (bf16/fp8 in, f32 accumulate),
    softmax/log-softmax in f32 on ACT/DVE.
  - Host does index preprocessing only: shard x, build per-dialogue 0/1
    adjacency masks from the edge lists, transpose/pack layouts, cast.
  - Emission is stage-major across the 4 dialogues so the PE never stalls
    on one dialogue's softmax chain; activations are function-major to
    avoid ACT LUT-table reloads; inputs move as a few large multi-dim-AP
    DMAs ordered by first use, and the PE runs dependency-free warm-up
    matmuls during the DMA lead-in to hold the HAM clock at 2.4 GHz.

kernel(**inputs) takes FULL inputs, runs 8-core SPMD via
bass_utils.run_bass_kernel_spmd, returns the FULL (8192, 7) f32 output.
"""

import numpy as np
import ml_dtypes

BF16 = ml_dtypes.bfloat16
FP8 = ml_dtypes.float8_e4m3

# Problem constants (hardcoded per contract)
B, L, D, H, R, NB, C = 32, 256, 1024, 128, 8, 30, 7
MEM = D + H            # 1152
N = B * L              # 8192
NCORES = 8
DPC = B // NCORES      # dialogues per core = 4
NLOC = DPC * L         # nodes per core = 1024
NT = NLOC // 128       # node tiles per core = 8
KT = D // 128          # contraction tiles over D = 8
MT = MEM // 128        # tiles over MEM = 9

_cache = {}


def _build_program(use_mask, halves, ablocks, bounds=None, stop_after=None):
    if bounds is None:
        bounds = (((0, L), (0, L)), ((0, L), (0, L)))
    at_bounds, bt_bounds = bounds
    import concourse.bacc as bacc
    import concourse.tile as tile
    import concourse.mybir as mybir
    import concourse.bass as bass
    from concourse.masks import make_identity

    dt = mybir.dt
    f32, bf16, fp8 = dt.float32, dt.bfloat16, dt.float8e4
    AX = mybir.AxisListType.X
    AF = mybir.ActivationFunctionType
    OP = mybir.AluOpType

    nc = bacc.Bacc("TRN2", target_bir_lowering=False, debug=False,
                   num_devices=NCORES)

    dram = nc.dram_tensor
    xt_d = dram("xt", [D, NLOC], bf16, kind="ExternalInput")        # x^T [d, n]
    wrel_d = dram("wrel", [D, R * H], bf16, kind="ExternalInput")   # [d, r*H+h]
    wr1_d = dram("wr1", [D, H], bf16, kind="ExternalInput")
    at_d = dram("at", [DPC, R, L, L], fp8, kind="ExternalInput")    # A^T (0/1)
    bt_d = dram("bt", [DPC, L, L], fp8, kind="ExternalInput")       # B^T (0/1)
    invd_d = dram("invd", [DPC, L], f32, kind="ExternalInput")      # 1/deg
    w2_d = dram("w2", [2, H, H], bf16, kind="ExternalInput")        # rel2, root2
    wt_d = dram("wt", [MEM, MEM], bf16, kind="ExternalInput")
    wlin_d = dram("wlin", [MEM, H], bf16, kind="ExternalInput")
    wfc_d = dram("wfc", [H, C], bf16, kind="ExternalInput")
    bias_d = dram("bias", [128, 12], f32, kind="ExternalInput")
    bfc_d = dram("bfc", [1, C], bf16, kind="ExternalInput")
    if use_mask:
        um_d = dram("um", [DPC, 2, L], f32, kind="ExternalInput")   # um2, um
    out_d = dram("out", [NLOC, C], f32, kind="ExternalOutput")

    with tile.TileContext(nc) as tc:
        from contextlib import ExitStack
        with ExitStack() as ctx:
            consts = ctx.enter_context(tc.tile_pool(name="consts", bufs=1))
            big = ctx.enter_context(tc.tile_pool(name="big", bufs=1))
            work = ctx.enter_context(tc.tile_pool(name="work", bufs=6))
            ps = ctx.enter_context(tc.tile_pool(name="ps", bufs=6, space="PSUM"))
            pst = ctx.enter_context(tc.tile_pool(name="pst", bufs=2, space="PSUM"))

            dma_a = nc.sync.dma_start      # queue A: PE-critical operands
            dma_b = nc.scalar.dma_start    # queue B: everything else
            mm = nc.tensor.matmul

            # ---- persistent operand loads (one DMA per tensor) ----
            xt = consts.tile([128, KT, NLOC], bf16)
            dma_a(out=xt, in_=xt_d[:].rearrange("(k p) n -> p k n", p=128))
            wrel = consts.tile([128, KT, R * H], bf16)
            for h2 in range(2):
                dma_a(out=wrel[:, :, h2 * 512:(h2 + 1) * 512],
                      in_=wrel_d[:, h2 * 512:(h2 + 1) * 512]
                      .rearrange("(k p) n -> p k n", p=128))
            wr1 = consts.tile([128, KT, H], bf16)
            dma_a(out=wr1, in_=wr1_d[:].rearrange("(k p) n -> p k n", p=128))
            wt = consts.tile([128, MT, MEM], bf16)
            dma_a(out=wt, in_=wt_d[:].rearrange("(m p) n -> p m n", p=128))
            at = consts.tile([128, DPC, R, 2, L], fp8)
            dma_a(out=at,
                  in_=at_d[:].rearrange("d r (st p) t -> p d r st t", p=128))
            bt = consts.tile([128, DPC, 2, L], fp8)
            dma_a(out=bt, in_=bt_d[:].rearrange("d (st p) t -> p d st t", p=128))
            wlin = consts.tile([128, MT, H], bf16)
            dma_a(out=wlin, in_=wlin_d[:].rearrange("(m p) n -> p m n", p=128))
            w2 = consts.tile([128, 2, H], bf16)
            dma_b(out=w2, in_=w2_d[:].rearrange("j p h -> p j h"))
            wfc = consts.tile([128, C], bf16)
            dma_b(out=wfc, in_=wfc_d[:])
            bias = consts.tile([128, 12], f32)
            dma_b(out=bias, in_=bias_d[:])
            bfc = consts.tile([1, C], bf16)
            dma_b(out=bfc, in_=bfc_d[:])
            ones_row = consts.tile([1, 128], bf16)
            nc.vector.memset(ones_row, 1.0)
            ident = consts.tile([128, 128], bf16)
            make_identity(nc, ident)
            # keep the PE busy (HAM warm) during the input-DMA lead-in;
            # `warm` psum is never read.
            warm_in = consts.tile([128, 128], bf16)
            nc.vector.memset(warm_in, 0.0)
            warm = ps.tile([128, 512], f32, tag="mm")
            for _ in range(160):
                mm(warm[:, :128], lhsT=warm_in, rhs=warm_in, start=True,
                   stop=True, skip_group_check=True)

            def bcast(dst, src_ap):
                bc = bass.AP(tensor=src_ap.tensor, offset=src_ap.offset,
                             ap=[[0, 128]] + list(src_ap.ap))
                nc.gpsimd.dma_start(out=dst, in_=bc)

            invd = consts.tile([128, DPC, L], f32)
            bcast(invd, invd_d[:])
            if use_mask:
                um = consts.tile([128, DPC, 2, L], f32)
                bcast(um, um_d[:])

            # ---- stage 1: xr[n, (r,h)] = x @ w_rel (all relations) ----
            xr = consts.tile([128, NT, R * H], bf16)
            for h2, i in sorted(
                    (h2, i) for i in range(NT) for h2 in halves[i]):
                p = ps.tile([128, 512], f32, tag="mm")
                for k in range(KT):
                    mm(p, lhsT=xt[:, k, i * 128:(i + 1) * 128],
                       rhs=wrel[:, k, h2 * 512:(h2 + 1) * 512],
                       start=(k == 0), stop=(k == KT - 1))
                nc.vector.tensor_copy(xr[:, i, h2 * 512:(h2 + 1) * 512], p)

            if stop_after == "xr":
                return _finish(nc)

            out1T = consts.tile([128, DPC, L], bf16)   # [h, dlg, n]
            out1 = consts.tile([128, NT, H], bf16)     # [n, h]
            out2T = consts.tile([128, DPC, L], bf16)
            out2 = consts.tile([128, NT, H], bf16)
            hidT = consts.tile([128, DPC, L], bf16)

            # ---- stage 2: out1^T = (sum_r xr_r^T A_r^T)*invd + root^T + b1
            for d in range(DPC):
                n0 = d * L
                pa = ps.tile([128, 512], f32, tag="mm")
                blocks = ablocks[d]
                for bi, (r, st) in enumerate(blocks):
                    # first block runs full width (zeroes the psum region);
                    # the rest crop to the mask's nonzero column band.
                    lo, hi = (0, L) if bi == 0 else at_bounds[st]
                    mm(pa[:, lo:hi],
                       lhsT=xr[:, 2 * d + st, r * H:(r + 1) * H],
                       rhs=at[:, d, r, st, lo:hi], start=(bi == 0),
                       stop=(bi == len(blocks) - 1), skip_group_check=True)
                agg = work.tile([128, L], f32, tag="agg")
                nc.vector.tensor_mul(agg, pa[:, :L], invd[:, d, :])
                pr = ps.tile([128, 512], f32, tag="mm")
                for k in range(KT):
                    mm(pr[:, :L], lhsT=wr1[:, k, :], rhs=xt[:, k, n0:n0 + L],
                       start=(k == 0), stop=(k == KT - 1))
                nc.vector.scalar_tensor_tensor(
                    out=out1T[:, d, :], in0=pr[:, :L], scalar=bias[:, 0:1],
                    in1=agg, op0=OP.add, op1=OP.add)
            for d in range(DPC):
                for st in range(2):
                    tp = pst.tile([128, 128], bf16, tag="tr")
                    nc.tensor.transpose(tp, out1T[:, d, st * 128:(st + 1) * 128], ident)
                    nc.vector.tensor_copy(out1[:, 2 * d + st, :], tp)

            # ---- stage 3: GraphConv layer 2 ----
            nbTs = []
            for d in range(DPC):
                p2 = ps.tile([128, 512], f32, tag="mm")
                for st in range(2):
                    lo, hi = (0, L) if st == 0 else bt_bounds[st]
                    mm(p2[:, lo:hi], lhsT=out1[:, 2 * d + st, :],
                       rhs=bt[:, d, st, lo:hi],
                       start=(st == 0), stop=(st == 1), skip_group_check=True)
                nbT = work.tile([128, L], bf16, tag="nbT")
                nc.vector.tensor_copy(nbT, p2[:, :L])
                nbTs.append(nbT)
            for d in range(DPC):
                p3 = ps.tile([128, 512], f32, tag="mm")
                mm(p3[:, :L], lhsT=w2[:, 0, :], rhs=nbTs[d], start=True, stop=False)
                mm(p3[:, :L], lhsT=w2[:, 1, :], rhs=out1T[:, d, :],
                   start=False, stop=True)
                nc.scalar.activation(out2T[:, d, :], p3[:, :L], AF.Identity,
                                     bias=bias[:, 1:2])
            for d in range(DPC):
                for st in range(2):
                    tp = pst.tile([128, 128], bf16, tag="tr")
                    nc.tensor.transpose(tp, out2T[:, d, st * 128:(st + 1) * 128], ident)
                    nc.vector.tensor_copy(out2[:, 2 * d + st, :], tp)

            if stop_after == "rgcn":
                return _finish(nc)

            # M^T / M tile accessors over MEM
            def rhs_MT(mt_i, d):
                if mt_i < KT:
                    return xt[:, mt_i, d * L:(d + 1) * L]
                return out2T[:, d, :]

            # ---- stage 5: Xc^T = w_t^T M^T + b_t ----
            XcTs = []
            for d in range(DPC):
                XcT = big.tile([128, MT, L], bf16, tag=f"XcT{d}")
                XcTs.append(XcT)
                for n2 in range(MT):
                    p4 = ps.tile([128, 512], f32, tag="mm")
                    for m in range(MT):
                        mm(p4[:, :L], lhsT=wt[:, m, n2 * 128:(n2 + 1) * 128],
                           rhs=rhs_MT(m, d), start=(m == 0), stop=(m == MT - 1))
                    nc.scalar.activation(XcT[:, n2, :], p4[:, :L], AF.Identity,
                                         bias=bias[:, 2 + n2:3 + n2])

            if stop_after == "xc":
                return _finish(nc)

            # ---- stage 6: scores -> tanh -> masked softmax -> alpha^T ----
            # function-major: all tanh, then all exp (one ACT table load each)
            zs, nmxs = {}, {}
            for d in range(DPC):
                for tt in range(2):
                    p5 = ps.tile([128, 512], f32, tag="mm")
                    for n2 in range(MT):
                        mm(p5[:, :L], lhsT=XcTs[d][:, n2, tt * 128:(tt + 1) * 128],
                           rhs=rhs_MT(n2, d), start=(n2 == 0), stop=(n2 == MT - 1))
                    z = big.tile([128, L], f32, tag=f"z{d}{tt}")
                    if use_mask:
                        nc.vector.tensor_mul(z, p5[:, :L], um[:, d, 0, :])
                        nc.scalar.activation(z, z, AF.Tanh)
                    else:
                        nc.scalar.activation(z, p5[:, :L], AF.Tanh)
                    nmx = work.tile([128, 1], f32, tag="nmx")
                    nc.vector.reduce_max(out=nmx, in_=z, axis=AX, negate=True)
                    zs[(d, tt)] = z
                    nmxs[(d, tt)] = nmx
            alfs = {}
            for d in range(DPC):
                for tt in range(2):
                    z, nmx = zs[(d, tt)], nmxs[(d, tt)]
                    ssum = work.tile([128, 1], f32, tag="ssum")
                    nc.scalar.activation(z, z, AF.Exp, bias=nmx, accum_out=ssum)
                    if use_mask:
                        nc.vector.tensor_mul(z, z, um[:, d, 1, :])
                        nc.vector.reduce_sum(out=ssum, in_=z, axis=AX)
                    rinv = work.tile([128, 1], f32, tag="rinv")
                    nc.vector.reciprocal(rinv, ssum)
                    alf = big.tile([128, L], bf16, tag=f"alf{d}{tt}")
                    nc.vector.tensor_scalar_mul(alf, z, rinv)
                    alfs[(d, tt)] = alf
            # ---- stage 6.5: G = M @ w_lin (att@w_lin reassociated; att is
            # never materialized: hidden = relu(alpha @ G + b_lin))
            Gs = {}
            for d in range(DPC):
                for st in range(2):
                    pg = ps.tile([128, 512], f32, tag="mm")
                    for m in range(MT):
                        mm(pg[:, :H],
                           lhsT=rhs_MT(m, d)[:, st * 128:(st + 1) * 128],
                           rhs=wlin[:, m, :], start=(m == 0), stop=(m == MT - 1))
                    G = big.tile([128, H], bf16, tag=f"G{d}{st}")
                    if st == 0:
                        nc.vector.tensor_copy(G, pg[:, :H])
                    else:
                        nc.scalar.copy(G, pg[:, :H])
                    Gs[(d, st)] = G

            alphaTs = []
            for d in range(DPC):
                alphaT = big.tile([128, 2, L], bf16, tag=f"alphaT{d}")
                alphaTs.append(alphaT)
                for tt in range(2):
                    for st in range(2):
                        tp = pst.tile([128, 128], bf16, tag="tr")
                        nc.tensor.transpose(
                            tp, alfs[(d, tt)][:, st * 128:(st + 1) * 128], ident)
                        nc.vector.tensor_copy(
                            alphaT[:, st, tt * 128:(tt + 1) * 128], tp)

            if stop_after == "scores":
                return _finish(nc)

            # ---- stage 7: hidden^T = relu(G^T @ alpha^T + b_lin) ----
            for d in range(DPC):
                p7 = ps.tile([128, 512], f32, tag="mm")
                for st in range(2):
                    mm(p7[:, :L], lhsT=Gs[(d, st)], rhs=alphaTs[d][:, st, :],
                       start=(st == 0), stop=(st == 1))
                nc.scalar.activation(hidT[:, d, :], p7[:, :L], AF.Relu,
                                     bias=bias[:, 11:12])

            if stop_after == "att":
                return _finish(nc)

            # ---- stage 8: logits + log_softmax (function-major) ----
            o_all = consts.tile([128, DPC, 2, 8], f32)
            nm7s, s7s = {}, {}
            for d in range(DPC):
                for tt in range(2):
                    p8 = ps.tile([128, 512], f32, tag="mm")
                    mm(p8[:, :C], lhsT=hidT[:, d, tt * 128:(tt + 1) * 128],
                       rhs=wfc, start=True, stop=False)
                    mm(p8[:, :C], lhsT=ones_row, rhs=bfc, start=False, stop=True)
                    nm7 = work.tile([128, 1], f32, tag=f"nm7_{d}{tt}")
                    nc.vector.reduce_max(out=nm7, in_=p8[:, :C], axis=AX, negate=True)
                    e7 = work.tile([128, 8], f32, tag="e7")
                    s7 = work.tile([128, 1], f32, tag=f"s7_{d}{tt}")
                    nc.scalar.activation(e7[:, :C], p8[:, :C], AF.Exp,
                                         bias=nm7, accum_out=s7)
                    nc.vector.tensor_scalar_add(o_all[:, d, tt, :C], p8[:, :C], nm7)
                    nm7s[(d, tt)], s7s[(d, tt)] = nm7, s7
            for d in range(DPC):
                for tt in range(2):
                    nm7, s7 = nm7s[(d, tt)], s7s[(d, tt)]
                    ls7 = work.tile([128, 1], f32, tag="ls7")
                    nc.scalar.activation(ls7, s7, AF.Ln)
                    nc.vector.tensor_scalar(
                        out=o_all[:, d, tt, :C], in0=o_all[:, d, tt, :C],
                        scalar1=ls7, scalar2=None, op0=OP.subtract)
            dma_a(out=out_d[:].rearrange("(d tt p) c -> p d tt c", d=DPC, tt=2),
                  in_=o_all[:, :, :, 0:C])

    return _finish(nc)


def _finish(nc):
    nc.compile()
    return nc


def prep_inputs(x, edge_src, edge_dst, edge_type, umask, basis, comp,
                w_root1, b1, w_rel2, b_rel2, w_root2, w_t, b_t,
                w_lin, b_lin, w_fc, b_fc):
    """Host-side sharding / layout prep.

    Returns (in_maps, use_mask, halves, ablocks, perm).
    Nodes are permuted within each dialogue so same-speaker nodes are
    contiguous; then each 128-node tile only needs the relation-half
    matching its speaker(s), and all-zero adjacency blocks are skipped.
    """
    x = np.asarray(x, np.float32)
    src = np.asarray(edge_src, np.int64)
    dst = np.asarray(edge_dst, np.int64)
    ety = np.asarray(edge_type, np.int64)
    umask = np.asarray(umask, np.float32)
    basis = np.asarray(basis, np.float32)
    comp = np.asarray(comp, np.float32)

    # dialogue-locality of edges (guaranteed by the windowed construction)
    g_s = src // L
    assert np.array_equal(g_s, dst // L), "edges must stay within a dialogue"

    # infer per-node speaker from edge types (etype = s_src*4 + s_dst*2 + dir);
    # fall back to identity permutation if inconsistent.
    # identity node order (keeps the +-window band structure of the masks,
    # which the device exploits by cropping mask-matmul free dims)
    perm = np.arange(N, dtype=np.int64)

    # w_rel[r] = sum_b comp[r,b] basis[b]  -> layout [d, r*H+h]
    w_rel = np.einsum('rb,bdh->rdh', comp, basis)
    wrel_layout = np.ascontiguousarray(
        w_rel.transpose(1, 0, 2).reshape(D, R * H)).astype(BF16)

    deg = np.bincount(dst, minlength=N).astype(np.float64)
    inv_deg = np.where(deg > 0, 1.0 / np.maximum(deg, 1), 0.0).astype(np.float32)

    g_s = src // L
    at_all = np.zeros((B, R, L, L), np.float32)   # [dlg, r, src, dst] 0/1
    ls, ld = src % L, dst % L
    np.add.at(at_all, (g_s, ety, ls, ld), 1.0)
    bt_all = np.zeros((B, L, L), np.float32)
    np.add.at(bt_all, (g_s, ls, ld), 1.0)

    use_mask = not bool(np.all(umask == 1.0))

    bias_pack = np.zeros((128, 12), np.float32)
    bias_pack[:, 0] = np.asarray(b1, np.float32)
    bias_pack[:, 1] = np.asarray(b_rel2, np.float32)
    bias_pack[:, 2:11] = np.asarray(b_t, np.float32).reshape(9, 128).T
    bias_pack[:, 11] = np.asarray(b_lin, np.float32)

    shared = {
        "wrel": wrel_layout,
        "wr1": np.asarray(w_root1, np.float32).astype(BF16),
        "w2": np.stack([np.asarray(w_rel2, np.float32),
                        np.asarray(w_root2, np.float32)]).astype(BF16),
        "wt": np.asarray(w_t, np.float32).astype(BF16),
        "wlin": np.asarray(w_lin, np.float32).astype(BF16),
        "wfc": np.asarray(w_fc, np.float32).astype(BF16),
        "bias": bias_pack,
        "bfc": np.asarray(b_fc, np.float32).reshape(1, C).astype(BF16),
    }

    # per-core tile structure: which relation-halves each node-tile needs,
    # and which (r, st) adjacency blocks are nonzero per dialogue.
    # NOTE: the program structure must be IDENTICAL across cores (one SPMD
    # NEFF), so take the union over cores per (tile, dialogue) position.
    halves = [(0, 1)] * NT
    def col_bounds(nzmask):
        cols = np.flatnonzero(nzmask)
        if cols.size == 0:
            return (0, L)
        return (int(cols[0]), int(cols[-1]) + 1)

    at_bounds = tuple(
        col_bounds(at_all[:, :, st * 128:(st + 1) * 128, :].any(axis=(0, 1, 2)))
        for st in range(2))
    bt_bounds = tuple(
        col_bounds(bt_all[:, st * 128:(st + 1) * 128, :].any(axis=(0, 1)))
        for st in range(2))

    ablocks = []
    for d in range(DPC):
        blk = []
        for r in range(R):
            for st in range(2):
                nz = False
                for c in range(NCORES):
                    g = c * DPC + d
                    if at_all[g, r, st * 128:(st + 1) * 128, :].any():
                        nz = True
                        break
                if nz:
                    blk.append((r, st))
        ablocks.append(tuple(blk))

    in_maps = []
    for c in range(NCORES):
        xl = x[c * NLOC:(c + 1) * NLOC]
        m = dict(shared)
        m["xt"] = np.ascontiguousarray(xl.T).astype(BF16)
        m["at"] = at_all[c * DPC:(c + 1) * DPC].astype(FP8)
        m["bt"] = bt_all[c * DPC:(c + 1) * DPC].astype(FP8)
        m["invd"] = inv_deg[c * NLOC:(c + 1) * NLOC].reshape(DPC, L)
        if use_mask:
            uml = umask[c * DPC:(c + 1) * DPC]   # (DPC, L)
            m["um"] = np.stack([uml * uml, uml], axis=1).astype(np.float32)
        in_maps.append(m)
    return in_maps, use_mask, tuple(halves), tuple(ablocks), perm, (at_bounds, bt_bounds)


_last_results = None


def kernel(**inputs):
    global _last_results
    from concourse.bass_utils import run_bass_kernel_spmd

    in_maps, use_mask, halves, ablocks, perm, bounds = prep_inputs(**inputs)
    key = (use_mask, halves, ablocks, bounds)
    if key not in _cache:
        _cache[key] = _build_program(use_mask, halves, ablocks, bounds)
    nc = _cache[key]
    res = run_bass_kernel_spmd(nc, in_maps, core_ids=list(range(NCORES)))
    _last_results = res
    out_p = np.concatenate([res.results[c]["out"] for c in range(NCORES)], axis=0)
    out = np.empty_like(out_p)
    out[perm] = out_p
    return out



# revision 6
# speedup vs baseline: 1.3840x; 1.3840x over previous
"""Trainium2 Bass kernel for nn_DialogueGCNModel (DialogueGCN forward).

Strategy (data-parallel over dialogues, 4 dialogues per core):
  - Edges never cross dialogues, so the RGCN scatter/gather runs as dense
    per-dialogue 0/1-adjacency matmuls.
  - All heavy GEMMs run in fp8 e4m3 with MatmulPerfMode.DoubleRow (two
    128-deep contraction subtiles per instruction = 2x bf16 throughput).
    Small weights are pre-scaled by powers of two on the host so their
    values sit in e4m3's normal range; the descales fold into the ACT/DVE
    psum->sbuf evacuations (activation scale, tensor_scalar mult) and into
    the host-precomputed 1/deg vector, so descaling costs zero extra ops.
  - M = [x | out2] is kept feature-major in fp8 and padded to 10 feature
    tiles (the pad tile is zero), so every matmul over MEM=1152 runs as 5
    DoubleRow pairs with no odd-tile remainder.
  - The matchatt/softmax/classifier chain is emitted per-dialogue and
    interleaved so dialogue d's softmax (ACT/DVE) overlaps dialogue d+1's
    Xc/scores matmuls (PE). tanh and exp live in the same ACT function
    table, so interleaving costs no table reloads; the single switch to
    the ln table happens once, at the very end.
  - psum->sbuf evacuations are spread across DVE / ACT / GPSIMD so no one
    elementwise engine gates the PE.

kernel(**inputs) takes FULL inputs, runs 8-core SPMD via
bass_utils.run_bass_kernel_spmd, returns the FULL (8192, 7) f32 output.
"""

import numpy as np
import ml_dtypes

BF16 = ml_dtypes.bfloat16
FP8 = ml_dtypes.float8_e4m3

# Problem constants (hardcoded per contract)
B, L, D, H, R, NB, C = 32, 256, 1024, 128, 8, 30, 7
MEM = D + H            # 1152
N = B * L              # 8192
NCORES = 8
DPC = B // NCORES      # dialogues per core = 4
NLOC = DPC * L         # nodes per core = 1024
NT = NLOC // 128       # node tiles per core = 8
KT = D // 128          # contraction tiles over D = 8
MT = MEM // 128        # tiles over MEM = 9
MTP = 10               # padded (even) feature tiles over MEM

# power-of-two pre-scales applied host-side before fp8 casts
S_WREL = 256.0         # w_rel entries ~2e-3: lift into e4m3 normal range
S_W = 32.0             # w_root1 / w_rel2 / w_root2 / w_t / w_lin (~2e-2)
S_ALF = 64.0           # alpha ~4e-3: lift out of e4m3 subnormals

_cache = {}


def _build_program(use_mask, biases_zero, warmup=40):
    import concourse.bacc as bacc
    import concourse.tile as tile
    import concourse.mybir as mybir
    import concourse.bass as bass
    from concourse.masks import make_identity

    dt = mybir.dt
    f32, bf16, fp8 = dt.float32, dt.bfloat16, dt.float8e4
    AX = mybir.AxisListType.X
    AF = mybir.ActivationFunctionType
    OP = mybir.AluOpType
    DR = mybir.MatmulPerfMode.DoubleRow

    nc = bacc.Bacc("TRN2", target_bir_lowering=False, debug=False,
                   num_devices=NCORES)

    dram = nc.dram_tensor
    xt_d = dram("xt", [D, NLOC], fp8, kind="ExternalInput")          # x^T
    wrel_d = dram("wrel", [D, R * H], fp8, kind="ExternalInput")     # *S_WREL
    wr1_d = dram("wr1", [D, H], fp8, kind="ExternalInput")           # *S_W
    at_d = dram("at", [DPC, R, L, L], fp8, kind="ExternalInput")     # A^T 0/1
    bt_d = dram("bt", [DPC, L, L], fp8, kind="ExternalInput")        # B^T 0/1
    invd_d = dram("invd", [DPC, L], bf16, kind="ExternalInput")      # 1/deg/S_WREL
    w2_d = dram("w2", [2, H, H], fp8, kind="ExternalInput")          # *S_W
    wt_d = dram("wt", [MTP * 128, MEM], fp8, kind="ExternalInput")   # *S_W, padded
    wlin_d = dram("wlin", [MTP * 128, H], fp8, kind="ExternalInput")  # *S_W, padded
    wfc_d = dram("wfc", [H, C], bf16, kind="ExternalInput")
    bias_d = dram("bias", [128, 13], f32, kind="ExternalInput")
    bfc_d = dram("bfc", [1, C], bf16, kind="ExternalInput")
    if use_mask:
        um_d = dram("um", [DPC, 2, L], f32, kind="ExternalInput")    # um^2, um
    out_d = dram("out", [NLOC, C], f32, kind="ExternalOutput")

    with tile.TileContext(nc) as tc:
        from contextlib import ExitStack
        with ExitStack() as ctx:
            consts = ctx.enter_context(tc.tile_pool(name="consts", bufs=1))
            big = ctx.enter_context(tc.tile_pool(name="big", bufs=1))
            work = ctx.enter_context(tc.tile_pool(name="work", bufs=6))
            ps = ctx.enter_context(tc.tile_pool(name="ps", bufs=6, space="PSUM"))
            pst = ctx.enter_context(tc.tile_pool(name="pst", bufs=2, space="PSUM"))

            dma_a = nc.sync.dma_start      # queue A: PE-critical operands
            dma_b = nc.gpsimd.dma_start    # queue B: everything else
            mm = nc.tensor.matmul

            # ---- persistent operand loads, ordered by first use ----
            wrel = consts.tile([128, KT, R, H], fp8)
            xt = consts.tile([128, KT, NLOC], fp8)
            dma_a(out=wrel[:, :, 0:4, :],
                  in_=wrel_d[:, 0:512].rearrange("(k p) n -> p k n", p=128))
            dma_a(out=xt[:, :, 0:512],
                  in_=xt_d[:, 0:512].rearrange("(k p) n -> p k n", p=128))
            dma_a(out=wrel[:, :, 4:8, :],
                  in_=wrel_d[:, 512:1024].rearrange("(k p) n -> p k n", p=128))
            dma_a(out=xt[:, :, 512:1024],
                  in_=xt_d[:, 512:1024].rearrange("(k p) n -> p k n", p=128))
            wr1 = consts.tile([128, KT, H], fp8)
            dma_a(out=wr1, in_=wr1_d[:].rearrange("(k p) n -> p k n", p=128))

            at = consts.tile([128, DPC, R, 2, L], fp8)
            dma_b(out=at,
                  in_=at_d[:].rearrange("d r (st p) t -> p d r st t", p=128))
            bt = consts.tile([128, DPC, 2, L], fp8)
            dma_b(out=bt, in_=bt_d[:].rearrange("d (st p) t -> p d st t", p=128))
            w2 = consts.tile([128, 2, H], fp8)
            dma_b(out=w2, in_=w2_d[:].rearrange("j p h -> p j h"))
            wt = consts.tile([128, MTP, MEM], fp8)
            dma_b(out=wt, in_=wt_d[:].rearrange("(m p) n -> p m n", p=128))
            wlin = consts.tile([128, MTP, H], fp8)
            dma_b(out=wlin, in_=wlin_d[:].rearrange("(m p) n -> p m n", p=128))
            wfc = consts.tile([128, C], bf16)
            dma_b(out=wfc, in_=wfc_d[:])
            bias = consts.tile([128, 13], f32)
            dma_b(out=bias, in_=bias_d[:])
            bfc = consts.tile([1, C], bf16)
            dma_b(out=bfc, in_=bfc_d[:])

            def bcast(dst, src_ap):
                bc = bass.AP(tensor=src_ap.tensor, offset=src_ap.offset,
                             ap=[[0, 128]] + list(src_ap.ap))
                nc.scalar.dma_start(out=dst, in_=bc)

            invd = consts.tile([128, DPC, L], bf16)
            bcast(invd, invd_d[:])
            if use_mask:
                um = consts.tile([128, DPC, 2, L], f32)
                bcast(um, um_d[:])

            ones_row = consts.tile([1, 128], bf16)
            nc.vector.memset(ones_row, 1.0)
            ident = consts.tile([128, 128], bf16)
            make_identity(nc, ident)

            # zero pads so every MEM contraction runs as 5 DoubleRow pairs
            out2T = consts.tile([128, DPC, 2, L], fp8)   # slot 1 stays zero
            nc.vector.memset(out2T, 0.0)
            XcT = consts.tile([128, DPC, MTP, L], fp8)   # slot 9 stays zero
            for d in range(DPC):
                nc.vector.memset(XcT[:, d, MT, :], 0.0)

            # hold the PE p-state clock up during the input-DMA lead-in;
            # `warm` psum is never read.
            warm_in = consts.tile([128, 128], bf16)
            nc.vector.memset(warm_in, 0.0)
            warm = ps.tile([128, 512], f32, tag="mm")
            for _ in range(warmup):
                mm(warm[:, :128], lhsT=warm_in, rhs=warm_in, start=True,
                   stop=True, skip_group_check=True)

            # ---- stage 1: xr[n, r, h] = x @ w_rel (all relations) ----
            # psum = S_WREL * true; stored at that scale in fp8.
            xr = consts.tile([128, NT, R, H], fp8)
            evac = [nc.vector, nc.scalar]
            for idx, (h2, i) in enumerate(
                    (h2, i) for h2 in range(2) for i in range(NT)):
                p = ps.tile([128, 512], f32, tag="mm")
                for kk in range(0, KT, 2):
                    mm(p, lhsT=xt[:, kk:kk + 2, i * 128:(i + 1) * 128],
                       rhs=wrel[:, kk:kk + 2, 4 * h2:4 * h2 + 4, :],
                       start=(kk == 0), stop=(kk == KT - 2), perf_mode=DR)
                eng = evac[idx % 2]
                if eng is nc.scalar:
                    eng.activation(xr[:, i, 4 * h2:4 * h2 + 4, :], p, AF.Identity)
                else:
                    eng.tensor_copy(xr[:, i, 4 * h2:4 * h2 + 4, :], p)

            out1T = consts.tile([128, DPC, L], bf16)   # [h, dlg, n]
            out1 = consts.tile([128, NT, H], fp8)      # [n, h]
            nbout = consts.tile([128, DPC, 2, L], fp8)
            hidT = consts.tile([128, DPC, L], bf16)

            # ---- stage 2+3: RGCN agg/root + GraphConv, per dialogue ----
            for d in range(DPC):
                n0 = d * L
                pa = ps.tile([128, 512], f32, tag="mm")
                for st in range(2):
                    for rp in range(0, R, 2):
                        mm(pa[:, :L], lhsT=xr[:, 2 * d + st, rp:rp + 2, :],
                           rhs=at[:, d, rp:rp + 2, st, :],
                           start=(st == 0 and rp == 0),
                           stop=(st == 1 and rp == R - 2),
                           perf_mode=DR, skip_group_check=True)
                agg = work.tile([128, L], f32, tag="agg")
                nc.vector.tensor_mul(agg, pa[:, :L], invd[:, d, :])
                pr = ps.tile([128, 512], f32, tag="mm")
                for kk in range(0, KT, 2):
                    mm(pr[:, :L], lhsT=wr1[:, kk:kk + 2, :],
                       rhs=xt[:, kk:kk + 2, n0:n0 + L],
                       start=(kk == 0), stop=(kk == KT - 2), perf_mode=DR)
                # out1 = root/S_W + agg  (bias[:,0] = 1/S_W)
                nc.vector.scalar_tensor_tensor(
                    out=out1T[:, d, :], in0=pr[:, :L], scalar=bias[:, 0:1],
                    in1=agg, op0=OP.mult, op1=OP.add)
                if not biases_zero:
                    nc.vector.tensor_scalar_add(out1T[:, d, :], out1T[:, d, :],
                                                bias[:, 1:2])
                for st in range(2):
                    tp = pst.tile([128, 128], bf16, tag="tr")
                    nc.tensor.transpose(tp, out1T[:, d, st * 128:(st + 1) * 128],
                                        ident)
                    nc.vector.tensor_copy(out1[:, 2 * d + st, :], tp)
                p2 = ps.tile([128, 512], f32, tag="mm")
                mm(p2[:, :L], lhsT=out1[:, 2 * d:2 * d + 2, :],
                   rhs=bt[:, d, 0:2, :], start=True, stop=True, perf_mode=DR,
                   skip_group_check=True)
                nc.vector.tensor_copy(nbout[:, d, 0, :], p2[:, :L])
                nc.gpsimd.tensor_copy(nbout[:, d, 1, :], out1T[:, d, :])
                p3 = ps.tile([128, 512], f32, tag="mm")
                mm(p3[:, :L], lhsT=w2[:, 0:2, :], rhs=nbout[:, d, 0:2, :],
                   start=True, stop=True, perf_mode=DR, skip_group_check=True)
                nc.scalar.activation(out2T[:, d, 0, :], p3[:, :L], AF.Identity,
                                     scale=1.0 / S_W, bias=bias[:, 2:3])

            # M^T feature-tile pair accessors (5 DoubleRow pairs over MEM)
            def m_pair(mp, d, lo=0, width=L):
                if mp < 4:
                    return xt[:, 2 * mp:2 * mp + 2, d * L + lo:d * L + lo + width]
                return out2T[:, d, 0:2, lo:lo + width]

            # ---- stages 5-8 interleaved per dialogue ----
            zs, nmxs, alfs = {}, {}, {}
            nm7s, s7s = {}, {}
            alphaT = consts.tile([128, DPC, 2, L], fp8)
            G8 = consts.tile([128, DPC, 2, H], fp8)
            o_all = consts.tile([128, DPC, 2, 8], f32)

            def emit_xc_scores(d):
                # Xc^T = (w_t^T M^T)/S_W + b_t, fp8
                for n2 in range(MT):
                    p4 = ps.tile([128, 512], f32, tag="mm")
                    for mp in range(5):
                        mm(p4[:, :L], lhsT=wt[:, 2 * mp:2 * mp + 2,
                                              n2 * 128:(n2 + 1) * 128],
                           rhs=m_pair(mp, d), start=(mp == 0), stop=(mp == 4),
                           perf_mode=DR)
                    eng = evac[n2 % 2]
                    if eng is nc.scalar:
                        eng.activation(XcT[:, d, n2, :], p4[:, :L], AF.Identity,
                                       scale=1.0 / S_W, bias=bias[:, 3 + n2:4 + n2])
                    else:
                        eng.tensor_scalar(
                            out=XcT[:, d, n2, :], in0=p4[:, :L],
                            scalar1=1.0 / S_W, scalar2=bias[:, 3 + n2:4 + n2],
                            op0=OP.mult, op1=OP.add)
                # scores + tanh + row-max
                for tt in range(2):
                    p5 = ps.tile([128, 512], f32, tag="mm")
                    for n2 in range(0, MTP, 2):
                        lhsT = XcT[:, d, n2:n2 + 2, tt * 128:(tt + 1) * 128]
                        rhs = m_pair(n2 // 2, d) if n2 < 8 else out2T[:, d, 0:2, :]
                        mm(p5[:, :L], lhsT=lhsT, rhs=rhs, start=(n2 == 0),
                           stop=(n2 == 8), perf_mode=DR)
                    z = big.tile([128, L], f32, tag=f"z{d}{tt}")
                    if use_mask:
                        nc.vector.tensor_mul(z, p5[:, :L], um[:, d, 0, :])
                        nc.scalar.activation(z, z, AF.Tanh)
                    else:
                        nc.scalar.activation(z, p5[:, :L], AF.Tanh)
                    nmx = work.tile([128, 1], f32, tag="nmx")
                    nc.vector.reduce_max(out=nmx, in_=z, axis=AX, negate=True)
                    zs[(d, tt)] = z
                    nmxs[(d, tt)] = nmx

            def emit_g(d):
                # G = (M @ w_lin)/S_W, fp8 (alpha @ G = att @ w_lin)
                for st in range(2):
                    pg = ps.tile([128, 512], f32, tag="mm")
                    for mp in range(5):
                        mm(pg[:, :H], lhsT=m_pair(mp, d, st * 128, 128),
                           rhs=wlin[:, 2 * mp:2 * mp + 2, :],
                           start=(mp == 0), stop=(mp == 4), perf_mode=DR)
                    nc.scalar.activation(G8[:, d, st, :], pg[:, :H],
                                         AF.Identity, scale=1.0 / S_W)

            def emit_softmax(d):
                # alf = S_ALF * softmax(tanh(scores)) in bf16
                for tt in range(2):
                    z, nmx = zs[(d, tt)], nmxs[(d, tt)]
                    ssum = work.tile([128, 1], f32, tag="ssum")
                    nc.scalar.activation(z, z, AF.Exp, bias=nmx, accum_out=ssum)
                    if use_mask:
                        nc.vector.tensor_mul(z, z, um[:, d, 1, :])
                        nc.vector.reduce_sum(out=ssum, in_=z, axis=AX)
                    rinv = work.tile([128, 1], f32, tag="rinv")
                    nc.vector.reciprocal(rinv, ssum)
                    nc.vector.tensor_scalar(out=rinv, in0=rinv, scalar1=S_ALF,
                                            scalar2=None, op0=OP.mult)
                    alf = big.tile([128, L], bf16, tag=f"alf{d}{tt}")
                    nc.vector.tensor_scalar_mul(alf, z, rinv)
                    alfs[(d, tt)] = alf

            def emit_att_cls(d):
                # alpha^T via PE transpose, hid^T = relu(G^T alpha^T / S_ALF)
                for tt in range(2):
                    for st in range(2):
                        tp = pst.tile([128, 128], bf16, tag="tr")
                        nc.tensor.transpose(
                            tp, alfs[(d, tt)][:, st * 128:(st + 1) * 128], ident)
                        nc.vector.tensor_copy(
                            alphaT[:, d, st, tt * 128:(tt + 1) * 128], tp)
                p7 = ps.tile([128, 512], f32, tag="mm")
                mm(p7[:, :L], lhsT=G8[:, d, 0:2, :], rhs=alphaT[:, d, 0:2, :],
                   start=True, stop=True, perf_mode=DR, skip_group_check=True)
                nc.scalar.activation(hidT[:, d, :], p7[:, :L], AF.Relu,
                                     scale=1.0 / S_ALF, bias=bias[:, 12:13])
                for tt in range(2):
                    p8 = ps.tile([128, 512], f32, tag="mm")
                    mm(p8[:, :C], lhsT=hidT[:, d, tt * 128:(tt + 1) * 128],
                       rhs=wfc, start=True, stop=False)
                    mm(p8[:, :C], lhsT=ones_row, rhs=bfc, start=False, stop=True)
                    nm7 = work.tile([128, 1], f32, tag=f"nm7_{d}{tt}")
                    nc.vector.reduce_max(out=nm7, in_=p8[:, :C], axis=AX,
                                         negate=True)
                    e7 = work.tile([128, 8], f32, tag="e7")
                    s7 = work.tile([128, 1], f32, tag=f"s7_{d}{tt}")
                    nc.scalar.activation(e7[:, :C], p8[:, :C], AF.Exp,
                                         bias=nm7, accum_out=s7)
                    nc.vector.tensor_scalar_add(o_all[:, d, tt, :C], p8[:, :C],
                                                nm7)
                    nm7s[(d, tt)], s7s[(d, tt)] = nm7, s7

            for d in range(DPC):
                emit_xc_scores(d)
                emit_g(d)
                emit_softmax(d)
                if d > 0:
                    emit_att_cls(d - 1)
            emit_att_cls(DPC - 1)

            # ---- final log-softmax correction (one ln-table switch) ----
            for d in range(DPC):
                for tt in range(2):
                    ls7 = work.tile([128, 1], f32, tag="ls7")
                    nc.scalar.activation(ls7, s7s[(d, tt)], AF.Ln)
                    nc.vector.tensor_scalar(
                        out=o_all[:, d, tt, :C], in0=o_all[:, d, tt, :C],
                        scalar1=ls7, scalar2=None, op0=OP.subtract)
            dma_a(out=out_d[:].rearrange("(d tt p) c -> p d tt c", d=DPC, tt=2),
                  in_=o_all[:, :, :, 0:C])

    nc.compile()
    return nc


def prep_inputs(x, edge_src, edge_dst, edge_type, umask, basis, comp,
                w_root1, b1, w_rel2, b_rel2, w_root2, w_t, b_t,
                w_lin, b_lin, w_fc, b_fc):
    """Host-side sharding / layout prep. Returns (in_maps, use_mask,
    biases_zero)."""
    x = np.asarray(x, np.float32)
    src = np.asarray(edge_src, np.int64)
    dst = np.asarray(edge_dst, np.int64)
    ety = np.asarray(edge_type, np.int64)
    umask = np.asarray(umask, np.float32)
    basis = np.asarray(basis, np.float32)
    comp = np.asarray(comp, np.float32)

    # dialogue-locality of edges (guaranteed by the windowed construction)
    g_s = src // L
    assert np.array_equal(g_s, dst // L), "edges must stay within a dialogue"

    w_rel = np.einsum('rb,bdh->rdh', comp, basis)
    wrel_layout = np.ascontiguousarray(
        (w_rel * S_WREL).transpose(1, 0, 2).reshape(D, R * H)).astype(FP8)

    deg = np.bincount(dst, minlength=N).astype(np.float64)
    inv_deg = np.where(deg > 0, 1.0 / np.maximum(deg, 1), 0.0)
    invd2 = (inv_deg / S_WREL).astype(BF16)

    at_all = np.zeros((B, R, L, L), np.float32)   # [dlg, r, src, dst] 0/1
    ls, ld = src % L, dst % L
    np.add.at(at_all, (g_s, ety, ls, ld), 1.0)
    bt_all = np.zeros((B, L, L), np.float32)
    np.add.at(bt_all, (g_s, ls, ld), 1.0)

    use_mask = not bool(np.all(umask == 1.0))
    b1 = np.asarray(b1, np.float32)
    b_rel2 = np.asarray(b_rel2, np.float32)
    b_t = np.asarray(b_t, np.float32)
    b_lin = np.asarray(b_lin, np.float32)
    biases_zero = bool(np.all(b1 == 0))

    bias_pack = np.zeros((128, 13), np.float32)
    bias_pack[:, 0] = 1.0 / S_W
    bias_pack[:, 1] = b1
    bias_pack[:, 2] = b_rel2
    bias_pack[:, 3:12] = b_t.reshape(9, 128).T
    bias_pack[:, 12] = b_lin

    def pad10(w):
        out = np.zeros((MTP * 128, w.shape[1]), np.float32)
        out[:MEM] = w
        return out

    shared = {
        "wrel": wrel_layout,
        "wr1": (np.asarray(w_root1, np.float32) * S_W).astype(FP8),
        "w2": (np.stack([np.asarray(w_rel2, np.float32),
                         np.asarray(w_root2, np.float32)]) * S_W).astype(FP8),
        "wt": pad10(np.asarray(w_t, np.float32) * S_W).astype(FP8),
        "wlin": pad10(np.asarray(w_lin, np.float32) * S_W).astype(FP8),
        "wfc": np.asarray(w_fc, np.float32).astype(BF16),
        "bias": bias_pack,
        "bfc": np.asarray(b_fc, np.float32).reshape(1, C).astype(BF16),
    }

    in_maps = []
    for c in range(NCORES):
        xl = x[c * NLOC:(c + 1) * NLOC]
        m = dict(shared)
        m["xt"] = np.ascontiguousarray(xl.T).astype(FP8)
        m["at"] = at_all[c * DPC:(c + 1) * DPC].astype(FP8)
        m["bt"] = bt_all[c * DPC:(c + 1) * DPC].astype(FP8)
        m["invd"] = invd2[c * NLOC:(c + 1) * NLOC].reshape(DPC, L)
        if use_mask:
            uml = umask[c * DPC:(c + 1) * DPC]   # (DPC, L)
            m["um"] = np.stack([uml * uml, uml], axis=1).astype(np.float32)
        in_maps.append(m)
    return in_maps, use_mask, biases_zero


_last_results = None


def kernel(**inputs):
    global _last_results
    from concourse.bass_utils import run_bass_kernel_spmd

    in_maps, use_mask, biases_zero = prep_inputs(**inputs)
    key = (use_mask, biases_zero)
    if key not in _cache:
        _cache[key] = _build_program(use_mask, biases_zero)
    nc = _cache[key]
    res = run_bass_kernel_spmd(nc, in_maps, core_ids=list(range(NCORES)))
    _last_results = res
    return np.concatenate([res.results[c]["out"] for c in range(NCORES)],
                          axis=0)
